# revision 12
# baseline (speedup 1.0000x reference)
"""GQA attention (B=2, S=2048, 16 Q heads / 8 KV heads, head_dim=128, RoPE,
no causal mask) on 8 Trainium2 NeuronCores.

Sharding: DP=2 on batch x TP=4 on heads. Each core computes 4 Q heads /
2 KV heads for one batch element, plus a row-sharded o_proj partial; the
host sums the 4 partials per batch (the "all-reduce").

v2: single fused pass, bf16 operands (f32 PSUM accumulation), everything
SBUF-resident (x loaded once, no Q/K/V DRAM round trips), h-major unit
order so early softmax exps overlap remaining Q projections, paired
[128,1024] exp activations, reciprocal_approx_fast for the softmax divide.
"""
import json
import math
from contextlib import ExitStack

import numpy as np

# ---------------------------------------------------------------------------
# Environment patches (required for the walrus build in this container)
# ---------------------------------------------------------------------------
_PATCHED = False


def _install_patches():
    """1) The walrus here rejects >1 sync wait per instruction; split extra
    waits onto single-wait NoOps inserted before the instruction (engines
    execute their stream in order, so semantics are preserved).
    2) antenv.axon_hooks is missing in this image; shim it so trace=True
    profiling works (used by test harnesses; harmless otherwise)."""
    global _PATCHED
    if _PATCHED:
        return
    _PATCHED = True

    import concourse.bass as bass

    counter = [0]

    def _split_multiwait(bir):
        for func in bir.get("functions", []):
            for block in func.get("blocks", []):
                new_insts = []
                for inst in block.get("instructions", []):
                    si = inst.get("sync_info")
                    waits = (si or {}).get("on_wait") or []
                    if len(waits) > 1:
                        for w in waits[:-1]:
                            counter[0] += 1
                            new_insts.append(
                                {
                                    "debug": inst.get("debug", 0),
                                    "engine": inst.get("engine"),
                                    "ins": [],
                                    "name": f"I-waitsplit-{counter[0]}",
                                    "opcode": "NoOp",
                                    "outs": [],
                                    "sync_info": {"on_wait": [w], "on_update": []},
                                }
                            )
                        si["on_wait"] = [waits[-1]]
                    new_insts.append(inst)
                block["instructions"] = new_insts
        return bir

    orig_to_json_bytes = bass.Bass.to_json_bytes

    def patched_to_json_bytes(self):
        bir = json.loads(orig_to_json_bytes(self))
        return json.dumps(_split_multiwait(bir)).encode()

    bass.Bass.to_json_bytes = patched_to_json_bytes

    # -- NTFF profile hook shim (for trace=True) --
    import sys
    import types

    if "antenv.axon_hooks" not in sys.modules:
        mod = types.ModuleType("antenv.axon_hooks")
        _hook = [None]
        try:
            from trn_agent_boot.trn_boot import _ntff_profile_via_ctypes

            _hook[0] = _ntff_profile_via_ctypes("/opt/axon/libaxon_pjrt.so")
        except Exception:
            pass
        mod.get_axon_ntff_profile_hook = lambda: _hook[0]
        mod.set_axon_ntff_profile_hook = lambda h: _hook.__setitem__(0, h)
        sys.modules["antenv.axon_hooks"] = mod

    # upload_artifacts needs external storage; make it a no-op locally.
    import concourse.bass_utils as bu

    bu.upload_artifacts = lambda tmpdir: str(tmpdir)


# ---------------------------------------------------------------------------
# Problem constants (hardcoded per contest contract)
# ---------------------------------------------------------------------------
B, S, HID = 2, 2048, 2048
N_HEADS, N_KV = 16, 8
HD = 128
TP = 4  # tensor-parallel factor over heads
NQ = N_HEADS // TP  # 4 q heads per core
NKV = N_KV // TP  # 2 kv heads per core
KT = HID // 128  # 16 contraction tiles
ST = S // 128  # 16 sequence tiles of 128
SC = 512  # free-dim chunk
NB = S // SC  # 4 chunks over S
SCALE = 1.0 / math.sqrt(HD)


def _build_nc():
    import concourse.bass as bass
    import concourse.tile as tile
    from concourse import mybir

    f32 = mybir.dt.float32
    bf16 = mybir.dt.bfloat16
    AF = mybir.ActivationFunctionType

    nc = bass.Bass()
    xT = nc.dram_tensor("xT", [HID, S], bf16, kind="ExternalInput")
    wq = nc.dram_tensor("wq", [HID, NQ * HD], bf16, kind="ExternalInput")
    wk = nc.dram_tensor("wk", [HID, NKV * HD], bf16, kind="ExternalInput")
    wv = nc.dram_tensor("wv", [HID, NKV * HD], bf16, kind="ExternalInput")
    wo = nc.dram_tensor("wo", [NQ * HD, HID], bf16, kind="ExternalInput")
    # cos2 rows = [cosT; sinT], sin2 rows = [sinT; cosT]  (stacked host-side)
    cos2 = nc.dram_tensor("cos2", [HD // 2, S], f32, kind="ExternalInput")
    sin2 = nc.dram_tensor("sin2", [HD // 2, S], f32, kind="ExternalInput")
    out = nc.dram_tensor("out", [S, HID], f32, kind="ExternalOutput")

    with tile.TileContext(nc) as tc, ExitStack() as ctx:
        # ---- persistent pools (whole kernel) ----
        const = ctx.enter_context(tc.tile_pool(name="const", bufs=1))
        keep = ctx.enter_context(tc.tile_pool(name="keep", bufs=1))

        ones_f = const.tile([128, 128], f32)
        nc.vector.memset(ones_f[:], 1.0)
        ones_mat = const.tile([128, 128], bf16)
        nc.vector.tensor_copy(ones_mat[:], ones_f[:])
        ones32 = const.tile([128, 32], bf16)
        nc.vector.tensor_copy(ones32[:], ones_f[:, 0:32])

        # persistent SBUF tensors
        kT_sb = [keep.tile([128, S], bf16, tag=f"kT{g}", name=f"kT{g}") for g in range(NKV)]
        qT_sb = [keep.tile([128, S], bf16, tag=f"qT{h}", name=f"qT{h}") for h in range(NQ)]
        v_sb = keep.tile([128, ST, NKV * HD], bf16, tag="v", name="v")
        oh_sb = [keep.tile([128, S], bf16, tag=f"oh{h}", name=f"oh{h}") for h in range(NQ)]

        # ================= Phase A: projections + RoPE =================
        with ExitStack() as actx:
            xpool = actx.enter_context(tc.tile_pool(name="xpool", bufs=1))
            wpool = actx.enter_context(tc.tile_pool(name="wpool", bufs=2))
            wkpool = actx.enter_context(tc.tile_pool(name="wkpool", bufs=1))
            aconst = actx.enter_context(tc.tile_pool(name="aconst", bufs=1))
            rstage = actx.enter_context(tc.tile_pool(name="rstage", bufs=3))
            pspj = actx.enter_context(tc.tile_pool(name="pspj", bufs=4, space="PSUM"))

            # ---- DMA loads (emission order = priority order) ----
            wk_sb = wkpool.tile([128, KT, NKV * HD], bf16, tag="wk")
            nc.gpsimd.dma_start(
                out=wk_sb[:], in_=wk.rearrange("(kt p) d -> p kt d", p=128)
            )
            x_sb = xpool.tile([128, KT, S], bf16, tag="x")
            x_re = xT.rearrange("(kt p) s -> p kt s", p=128)
            for kq in range(4):
                nc.sync.dma_start(
                    out=x_sb[:, 4 * kq : 4 * kq + 4, 0:SC],
                    in_=x_re[:, 4 * kq : 4 * kq + 4, 0:SC],
                )
            cos_sb = aconst.tile([HD // 2, S], f32)
            sin_sb = aconst.tile([HD // 2, S], f32)
            nc.sync.dma_start(out=cos_sb[:], in_=cos2[:, :])
            nc.sync.dma_start(out=sin_sb[:], in_=sin2[:, :])
            for c in range(1, NB):
                nc.sync.dma_start(
                    out=x_sb[:, :, c * SC : (c + 1) * SC],
                    in_=x_re[:, :, c * SC : (c + 1) * SC],
                )
            wv_sb = aconst.tile([128, KT, NKV * HD], bf16)
            nc.sync.dma_start(
                out=wv_sb[:], in_=wv.rearrange("(kt p) d -> p kt d", p=128)
            )
            wq_re = wq.rearrange("(kt p) (h d) -> p kt h d", p=128, d=HD)
            wq_tiles = {}

            def load_wq(h):
                wq_tiles[h] = wpool.tile([128, KT, HD], bf16, tag="wq", name=f"wq{h}")
                nc.sync.dma_start(out=wq_tiles[h][:], in_=wq_re[:, :, h, :])

            load_wq(0)
            load_wq(1)

            # ---- RoPE: ps [128(re/im),512] -> dst[:, c-slice] ----
            # (PSUM inputs may cross partition bases; SBUF+SBUF may not)
            def rope(ps, dst, c0):
                re = ps[0:64, :]
                im = ps[64:128, :]
                cs = cos_sb[:, c0 : c0 + SC]
                sn = sin_sb[:, c0 : c0 + SC]
                t1 = rstage.tile([64, SC], f32, tag="t1")
                t2 = rstage.tile([64, SC], f32, tag="t2")
                t3 = rstage.tile([64, SC], f32, tag="t3")
                t4 = rstage.tile([64, SC], f32, tag="t4")
                nc.vector.tensor_mul(t1[:], re, cs)
                nc.vector.tensor_mul(t2[:], im, sn)
                nc.gpsimd.tensor_sub(dst[0:64, c0 : c0 + SC], t1[:], t2[:])
                nc.vector.tensor_mul(t3[:], re, sn)
                nc.vector.tensor_mul(t4[:], im, cs)
                nc.gpsimd.tensor_add(dst[64:128, c0 : c0 + SC], t3[:], t4[:])

            # ---- K projection + RoPE (all S), then V projection ----
            for c in range(NB):
                c0 = c * SC
                for g in range(NKV):
                    ps = pspj.tile([128, SC], f32, tag="pj")
                    for kt in range(KT):
                        nc.tensor.matmul(
                            ps[:],
                            wk_sb[:, kt, g * HD : (g + 1) * HD],
                            x_sb[:, kt, c0 : c0 + SC],
                            start=(kt == 0),
                            stop=(kt == KT - 1),
                        )
                    rope(ps, kT_sb[g], c0)
            for st in range(ST):
                ps = pspj.tile([128, SC], f32, tag="pj")
                for kt in range(KT):
                    nc.tensor.matmul(
                        ps[:, 0 : NKV * HD],
                        x_sb[:, kt, st * 128 : (st + 1) * 128],
                        wv_sb[:, kt, :],
                        start=(kt == 0),
                        stop=(kt == KT - 1),
                    )
                nc.vector.tensor_copy(v_sb[:, st, :], ps[:, 0 : NKV * HD])

            # ---- Q projection + RoPE, per head ----
            for h in range(NQ):
                if h >= 2:
                    load_wq(h)
                for c in range(NB):
                    c0 = c * SC
                    ps = pspj.tile([128, SC], f32, tag="pj")
                    for kt in range(KT):
                        nc.tensor.matmul(
                            ps[:],
                            wq_tiles[h][:, kt, :],
                            x_sb[:, kt, c0 : c0 + SC],
                            start=(kt == 0),
                            stop=(kt == KT - 1),
                        )
                    rope(ps, qT_sb[h], c0)

        # ================= Phase B: attention units + o_proj =================
        with ExitStack() as bctx:
            bpool = bctx.enter_context(tc.tile_pool(name="bpool", bufs=1))
            btmp = bctx.enter_context(tc.tile_pool(name="btmp", bufs=2))
            ptpool = bctx.enter_context(tc.tile_pool(name="ptpool", bufs=2))
            uctx = ExitStack()
            # PSUM: psmm 2x[128,1024](8KB/p) + pspv 2KB + psden 2KB = 12KB
            psmm = uctx.enter_context(tc.tile_pool(name="psmm", bufs=3, space="PSUM"))
            pspv = uctx.enter_context(tc.tile_pool(name="pspv", bufs=1, space="PSUM"))
            psden = uctx.enter_context(tc.tile_pool(name="psden", bufs=1, space="PSUM"))

            wo_sb = bpool.tile([128, NQ, HID], bf16)
            nc.sync.dma_start(
                out=wo_sb[:], in_=wo.rearrange("(h p) n -> p h n", p=128)
            )

            # ---- attention unit: (h, nb) -> oh_sb[h][:, nb*SC:...] ----
            def unit(h, nb):
                g = h // (NQ // NKV)
                c0 = nb * SC
                pt = ptpool.tile([128, KT, SC], bf16, tag="pt")
                # scores in pairs: two N=512 matmul groups into one
                # [128,1024] psum tile, one exp ACTIVATE per pair
                for p in range(KT // 2):
                    pss = psmm.tile([128, 2 * SC], f32, tag="mm")
                    for j in range(2):
                        kt = 2 * p + j
                        nc.tensor.matmul(
                            pss[:, j * SC : (j + 1) * SC],
                            kT_sb[g][:, kt * 128 : (kt + 1) * 128],
                            qT_sb[h][:, c0 : c0 + SC],
                            start=True,
                            stop=True,
                        )
                    nc.scalar.activation(
                        pt[:, 2 * p : 2 * p + 2, :], pss[:], AF.Exp, scale=SCALE
                    )
                # denominator (broadcast over partitions via ones matmul)
                psdb = psden.tile([128, SC], f32, tag="db")
                for kt in range(KT):
                    nc.tensor.matmul(
                        psdb[:],
                        ones_mat[:],
                        pt[:, kt, :],
                        start=(kt == 0),
                        stop=(kt == KT - 1),
                    )
                recb = btmp.tile([128, SC], f32, tag="recb")
                nc.vector.reciprocal(recb[:], psdb[:])
                # PV
                ppv = pspv.tile([128, SC], f32, tag="pv")
                for kt in range(KT):
                    nc.tensor.matmul(
                        ppv[:],
                        v_sb[:, kt, g * HD : (g + 1) * HD],
                        pt[:, kt, :],
                        start=(kt == 0),
                        stop=(kt == KT - 1),
                    )
                nc.vector.tensor_mul(oh_sb[h][:, c0 : c0 + SC], ppv[:], recb[:])

            for h in range(NQ):
                for nb in range(NB):
                    unit(h, nb)

            uctx.close()
            # ---- o_proj: out[st,:] += sum_h oh[h][:,st].T @ wo[h] ----
            # full-row [128,S] PSUM accumulation: one evict + one 1MB DMA
            # per st-tile, PE-bound instead of eviction-throttled
            psop = bctx.enter_context(tc.tile_pool(name="psop", bufs=2, space="PSUM"))
            for st in range(ST):
                pso = psop.tile([128, S], f32, tag="op")
                for nn in range(NB):
                    for h in range(NQ):
                        nc.tensor.matmul(
                            pso[:, nn * SC : (nn + 1) * SC],
                            oh_sb[h][:, st * 128 : (st + 1) * 128],
                            wo_sb[:, h, nn * SC : (nn + 1) * SC],
                            start=(h == 0),
                            stop=(h == NQ - 1),
                        )
                ot = btmp.tile([128, S], f32, tag="ost")
                nc.scalar.copy(ot[:], pso[:])
                nc.sync.dma_start(
                    out=out[st * 128 : (st + 1) * 128, :],
                    in_=ot[:],
                )
    return nc


_NC_CACHE = None


def _get_nc():
    global _NC_CACHE
    if _NC_CACHE is None:
        _install_patches()
        _NC_CACHE = _build_nc()
    return _NC_CACHE


# De-interleave permutation: within each head, even dims then odd dims.
_PERM = np.concatenate([np.arange(0, HD, 2), np.arange(1, HD, 2)])

_last_in_maps = None


def kernel(x, Wq, Wk, Wv, Wo, freqs_cos, freqs_sin, start_pos):
    _install_patches()
    import ml_dtypes

    from concourse.bass_utils import run_bass_kernel_spmd

    bf16 = ml_dtypes.bfloat16
    x = np.asarray(x, dtype=np.float32)
    Wq = np.asarray(Wq, dtype=np.float32)
    Wk = np.asarray(Wk, dtype=np.float32)
    Wv = np.asarray(Wv, dtype=np.float32)
    Wo = np.asarray(Wo, dtype=np.float32)
    cos2 = np.ascontiguousarray(np.asarray(freqs_cos, dtype=np.float32).T)
    sin2 = np.ascontiguousarray(np.asarray(freqs_sin, dtype=np.float32).T)

    # Per-head de-interleave of Wq/Wk columns (RoPE pairs -> [re, im] blocks)
    Wq_p = Wq.reshape(HID, N_HEADS, HD)[:, :, _PERM]
    Wk_p = Wk.reshape(HID, N_KV, HD)[:, :, _PERM]

    in_maps = []
    for core in range(8):
        b, t = divmod(core, TP)
        xT_b = np.ascontiguousarray(x[b].T.astype(bf16))
        wq_c = np.ascontiguousarray(
            Wq_p[:, t * NQ : (t + 1) * NQ, :].reshape(HID, NQ * HD).astype(bf16)
        )
        wk_c = np.ascontiguousarray(
            Wk_p[:, t * NKV : (t + 1) * NKV, :].reshape(HID, NKV * HD).astype(bf16)
        )
        wv_c = np.ascontiguousarray(
            Wv.reshape(HID, N_KV, HD)[:, t * NKV : (t + 1) * NKV, :]
            .reshape(HID, NKV * HD)
            .astype(bf16)
        )
        wo_c = np.ascontiguousarray(
            Wo[t * NQ * HD : (t + 1) * NQ * HD, :].astype(bf16)
        )
        in_maps.append(
            {
                "xT": xT_b,
                "wq": wq_c,
                "wk": wk_c,
                "wv": wv_c,
                "wo": wo_c,
                "cos2": cos2,
                "sin2": sin2,
            }
        )

    global _last_in_maps
    _last_in_maps = in_maps
    nc = _get_nc()
    res = run_bass_kernel_spmd(nc, in_maps, list(range(8)))
    outs = [res.results[c]["out"] for c in range(8)]
    full = np.stack(
        [sum(outs[b * TP + t] for t in range(TP)) for b in range(B)]
    ).astype(np.float32)
    return full


# revision 13
# speedup vs baseline: 1.0174x; 1.0174x over previous
"""GQA attention (B=2, S=2048, 16 Q heads / 8 KV heads, head_dim=128, RoPE,
no causal mask) on 8 Trainium2 NeuronCores.

Sharding: DP=2 on batch x TP=4 on heads. Each core computes 4 Q heads /
2 KV heads for one batch element, plus a row-sharded o_proj partial; the
host sums the 4 partials per batch (the "all-reduce").

v2: single fused pass, bf16 operands (f32 PSUM accumulation), everything
SBUF-resident (x loaded once, no Q/K/V DRAM round trips), h-major unit
order so early softmax exps overlap remaining Q projections, paired
[128,1024] exp activations, reciprocal_approx_fast for the softmax divide.
"""
import json
import math
from contextlib import ExitStack

import numpy as np

# ---------------------------------------------------------------------------
# Environment patches (required for the walrus build in this container)
# ---------------------------------------------------------------------------
_PATCHED = False


def _install_patches():
    """1) The walrus here rejects >1 sync wait per instruction; split extra
    waits onto single-wait NoOps inserted before the instruction (engines
    execute their stream in order, so semantics are preserved).
    2) antenv.axon_hooks is missing in this image; shim it so trace=True
    profiling works (used by test harnesses; harmless otherwise)."""
    global _PATCHED
    if _PATCHED:
        return
    _PATCHED = True

    import concourse.bass as bass

    counter = [0]

    def _split_multiwait(bir):
        for func in bir.get("functions", []):
            for block in func.get("blocks", []):
                new_insts = []
                for inst in block.get("instructions", []):
                    si = inst.get("sync_info")
                    waits = (si or {}).get("on_wait") or []
                    if len(waits) > 1:
                        for w in waits[:-1]:
                            counter[0] += 1
                            new_insts.append(
                                {
                                    "debug": inst.get("debug", 0),
                                    "engine": inst.get("engine"),
                                    "ins": [],
                                    "name": f"I-waitsplit-{counter[0]}",
                                    "opcode": "NoOp",
                                    "outs": [],
                                    "sync_info": {"on_wait": [w], "on_update": []},
                                }
                            )
                        si["on_wait"] = [waits[-1]]
                    new_insts.append(inst)
                block["instructions"] = new_insts
        return bir

    orig_to_json_bytes = bass.Bass.to_json_bytes

    def patched_to_json_bytes(self):
        bir = json.loads(orig_to_json_bytes(self))
        return json.dumps(_split_multiwait(bir)).encode()

    bass.Bass.to_json_bytes = patched_to_json_bytes

    # -- NTFF profile hook shim (for trace=True) --
    import sys
    import types

    if "antenv.axon_hooks" not in sys.modules:
        mod = types.ModuleType("antenv.axon_hooks")
        _hook = [None]
        try:
            from trn_agent_boot.trn_boot import _ntff_profile_via_ctypes

            _hook[0] = _ntff_profile_via_ctypes("/opt/axon/libaxon_pjrt.so")
        except Exception:
            pass
        mod.get_axon_ntff_profile_hook = lambda: _hook[0]
        mod.set_axon_ntff_profile_hook = lambda h: _hook.__setitem__(0, h)
        sys.modules["antenv.axon_hooks"] = mod

    # upload_artifacts needs external storage; make it a no-op locally.
    import concourse.bass_utils as bu

    bu.upload_artifacts = lambda tmpdir: str(tmpdir)


# ---------------------------------------------------------------------------
# Problem constants (hardcoded per contest contract)
# ---------------------------------------------------------------------------
B, S, HID = 2, 2048, 2048
N_HEADS, N_KV = 16, 8
HD = 128
TP = 4  # tensor-parallel factor over heads
NQ = N_HEADS // TP  # 4 q heads per core
NKV = N_KV // TP  # 2 kv heads per core
KT = HID // 128  # 16 contraction tiles
ST = S // 128  # 16 sequence tiles of 128
SC = 512  # free-dim chunk
NB = S // SC  # 4 chunks over S
SCALE = 1.0 / math.sqrt(HD)


def _build_nc():
    import concourse.bass as bass
    import concourse.tile as tile
    from concourse import mybir

    f32 = mybir.dt.float32
    bf16 = mybir.dt.bfloat16
    AF = mybir.ActivationFunctionType

    nc = bass.Bass()
    xT = nc.dram_tensor("xT", [HID, S], bf16, kind="ExternalInput")
    wq = nc.dram_tensor("wq", [HID, NQ * HD], bf16, kind="ExternalInput")
    wk = nc.dram_tensor("wk", [HID, NKV * HD], bf16, kind="ExternalInput")
    wv = nc.dram_tensor("wv", [HID, NKV * HD], bf16, kind="ExternalInput")
    wo = nc.dram_tensor("wo", [NQ * HD, HID], bf16, kind="ExternalInput")
    # cos2 rows = [cosT; sinT], sin2 rows = [sinT; cosT]  (stacked host-side)
    cos2 = nc.dram_tensor("cos2", [HD // 2, S], f32, kind="ExternalInput")
    sin2 = nc.dram_tensor("sin2", [HD // 2, S], f32, kind="ExternalInput")
    out = nc.dram_tensor("out", [S, HID], f32, kind="ExternalOutput")

    with tile.TileContext(nc) as tc, ExitStack() as ctx:
        # ---- persistent pools (whole kernel) ----
        const = ctx.enter_context(tc.tile_pool(name="const", bufs=1))
        keep = ctx.enter_context(tc.tile_pool(name="keep", bufs=1))

        ones_f = const.tile([128, 128], f32)
        nc.vector.memset(ones_f[:], 1.0)
        ones_mat = const.tile([128, 128], bf16)
        nc.vector.tensor_copy(ones_mat[:], ones_f[:])
        ones32 = const.tile([128, 32], bf16)
        nc.vector.tensor_copy(ones32[:], ones_f[:, 0:32])

        # persistent SBUF tensors
        kT_sb = [keep.tile([128, S], bf16, tag=f"kT{g}", name=f"kT{g}") for g in range(NKV)]
        qT_sb = [keep.tile([128, S], bf16, tag=f"qT{h}", name=f"qT{h}") for h in range(NQ)]
        v_sb = keep.tile([128, ST, NKV * HD], bf16, tag="v", name="v")
        oh_sb = [keep.tile([128, S], bf16, tag=f"oh{h}", name=f"oh{h}") for h in range(NQ)]

        # ================= Phase A: projections + RoPE =================
        with ExitStack() as actx:
            xpool = actx.enter_context(tc.tile_pool(name="xpool", bufs=1))
            wpool = actx.enter_context(tc.tile_pool(name="wpool", bufs=2))
            wkpool = actx.enter_context(tc.tile_pool(name="wkpool", bufs=1))
            aconst = actx.enter_context(tc.tile_pool(name="aconst", bufs=1))
            rstage = actx.enter_context(tc.tile_pool(name="rstage", bufs=3))
            pspj = actx.enter_context(tc.tile_pool(name="pspj", bufs=4, space="PSUM"))

            # ---- DMA loads (emission order = priority order) ----
            wk_sb = wkpool.tile([128, KT, NKV * HD], bf16, tag="wk")
            nc.gpsimd.dma_start(
                out=wk_sb[:], in_=wk.rearrange("(kt p) d -> p kt d", p=128)
            )
            x_sb = xpool.tile([128, KT, S], bf16, tag="x")
            x_re = xT.rearrange("(kt p) s -> p kt s", p=128)
            for kq in range(4):
                nc.sync.dma_start(
                    out=x_sb[:, 4 * kq : 4 * kq + 4, 0:SC],
                    in_=x_re[:, 4 * kq : 4 * kq + 4, 0:SC],
                )
            cos_sb = aconst.tile([HD // 2, S], f32)
            sin_sb = aconst.tile([HD // 2, S], f32)
            nc.sync.dma_start(out=cos_sb[:], in_=cos2[:, :])
            nc.sync.dma_start(out=sin_sb[:], in_=sin2[:, :])
            for c in range(1, NB):
                nc.sync.dma_start(
                    out=x_sb[:, :, c * SC : (c + 1) * SC],
                    in_=x_re[:, :, c * SC : (c + 1) * SC],
                )
            wv_sb = aconst.tile([128, KT, NKV * HD], bf16)
            nc.sync.dma_start(
                out=wv_sb[:], in_=wv.rearrange("(kt p) d -> p kt d", p=128)
            )
            wq_re = wq.rearrange("(kt p) (h d) -> p kt h d", p=128, d=HD)
            wq_tiles = {}

            def load_wq(h):
                wq_tiles[h] = wpool.tile([128, KT, HD], bf16, tag="wq", name=f"wq{h}")
                nc.sync.dma_start(out=wq_tiles[h][:], in_=wq_re[:, :, h, :])

            load_wq(0)
            load_wq(1)

            # ---- RoPE: ps [128(re/im),512] -> dst[:, c-slice] ----
            # (PSUM inputs may cross partition bases; SBUF+SBUF may not)
            def rope(ps, dst, c0):
                re = ps[0:64, :]
                im = ps[64:128, :]
                cs = cos_sb[:, c0 : c0 + SC]
                sn = sin_sb[:, c0 : c0 + SC]
                t1 = rstage.tile([64, SC], f32, tag="t1")
                t2 = rstage.tile([64, SC], f32, tag="t2")
                t3 = rstage.tile([64, SC], f32, tag="t3")
                t4 = rstage.tile([64, SC], f32, tag="t4")
                nc.vector.tensor_mul(t1[:], re, cs)
                nc.vector.tensor_mul(t2[:], im, sn)
                nc.gpsimd.tensor_sub(dst[0:64, c0 : c0 + SC], t1[:], t2[:])
                nc.vector.tensor_mul(t3[:], re, sn)
                nc.vector.tensor_mul(t4[:], im, cs)
                nc.gpsimd.tensor_add(dst[64:128, c0 : c0 + SC], t3[:], t4[:])

            # ---- K projection + RoPE (all S), then V projection ----
            for c in range(NB):
                c0 = c * SC
                for g in range(NKV):
                    ps = pspj.tile([128, SC], f32, tag="pj")
                    for kt in range(KT):
                        nc.tensor.matmul(
                            ps[:],
                            wk_sb[:, kt, g * HD : (g + 1) * HD],
                            x_sb[:, kt, c0 : c0 + SC],
                            start=(kt == 0),
                            stop=(kt == KT - 1),
                        )
                    rope(ps, kT_sb[g], c0)
            for st in range(ST):
                ps = pspj.tile([128, SC], f32, tag="pj")
                for kt in range(KT):
                    nc.tensor.matmul(
                        ps[:, 0 : NKV * HD],
                        x_sb[:, kt, st * 128 : (st + 1) * 128],
                        wv_sb[:, kt, :],
                        start=(kt == 0),
                        stop=(kt == KT - 1),
                    )
                nc.vector.tensor_copy(v_sb[:, st, :], ps[:, 0 : NKV * HD])

            # ---- Q projection + RoPE, per head ----
            for h in range(NQ):
                if h >= 2:
                    load_wq(h)
                for c in range(NB):
                    c0 = c * SC
                    ps = pspj.tile([128, SC], f32, tag="pj")
                    for kt in range(KT):
                        nc.tensor.matmul(
                            ps[:],
                            wq_tiles[h][:, kt, :],
                            x_sb[:, kt, c0 : c0 + SC],
                            start=(kt == 0),
                            stop=(kt == KT - 1),
                        )
                    rope(ps, qT_sb[h], c0)

        # ================= Phase B: attention units + o_proj =================
        with ExitStack() as bctx:
            bpool = bctx.enter_context(tc.tile_pool(name="bpool", bufs=1))
            btmp = bctx.enter_context(tc.tile_pool(name="btmp", bufs=2))
            ptpool = bctx.enter_context(tc.tile_pool(name="ptpool", bufs=2))
            uctx = ExitStack()
            # PSUM: psmm 2x[128,1024](8KB/p) + pspv 2KB + psden 2KB = 12KB
            psmm = uctx.enter_context(tc.tile_pool(name="psmm", bufs=2, space="PSUM"))
            pspv = uctx.enter_context(tc.tile_pool(name="pspv", bufs=1, space="PSUM"))
            psden = uctx.enter_context(tc.tile_pool(name="psden", bufs=1, space="PSUM"))

            wo_sb = bpool.tile([128, NQ, HID], bf16)
            nc.sync.dma_start(
                out=wo_sb[:], in_=wo.rearrange("(h p) n -> p h n", p=128)
            )

            # ---- attention unit: (h, nb) -> oh_sb[h][:, nb*SC:...] ----
            def unit(h, nb):
                g = h // (NQ // NKV)
                c0 = nb * SC
                pt = ptpool.tile([128, KT, SC], bf16, tag="pt")
                # scores in pairs: two N=512 matmul groups into one
                # [128,1024] psum tile, one exp ACTIVATE per pair
                for p in range(KT // 2):
                    pss = psmm.tile([128, 2 * SC], f32, tag="mm")
                    for j in range(2):
                        kt = 2 * p + j
                        nc.tensor.matmul(
                            pss[:, j * SC : (j + 1) * SC],
                            kT_sb[g][:, kt * 128 : (kt + 1) * 128],
                            qT_sb[h][:, c0 : c0 + SC],
                            start=True,
                            stop=True,
                        )
                    nc.scalar.activation(
                        pt[:, 2 * p : 2 * p + 2, :], pss[:], AF.Exp, scale=SCALE
                    )
                # denominator (broadcast over partitions via ones matmul)
                psdb = psden.tile([128, SC], f32, tag="db")
                for kt in range(KT):
                    nc.tensor.matmul(
                        psdb[:],
                        ones_mat[:],
                        pt[:, kt, :],
                        start=(kt == 0),
                        stop=(kt == KT - 1),
                    )
                recb = btmp.tile([128, SC], f32, tag="recb")
                nc.vector.reciprocal(recb[:], psdb[:])
                # PV
                ppv = pspv.tile([128, SC], f32, tag="pv")
                for kt in range(KT):
                    nc.tensor.matmul(
                        ppv[:],
                        v_sb[:, kt, g * HD : (g + 1) * HD],
                        pt[:, kt, :],
                        start=(kt == 0),
                        stop=(kt == KT - 1),
                    )
                nc.vector.tensor_mul(oh_sb[h][:, c0 : c0 + SC], ppv[:], recb[:])

            for h in range(NQ):
                for nb in range(NB):
                    unit(h, nb)

            uctx.close()
            # ---- o_proj: out[st,:] += sum_h oh[h][:,st].T @ wo[h] ----
            # full-row [128,S] PSUM accumulation: one evict + one 1MB DMA
            # per st-tile, PE-bound instead of eviction-throttled
            psop = bctx.enter_context(tc.tile_pool(name="psop", bufs=2, space="PSUM"))
            for st in range(ST):
                pso = psop.tile([128, S], f32, tag="op")
                for nn in range(NB):
                    for h in range(NQ):
                        nc.tensor.matmul(
                            pso[:, nn * SC : (nn + 1) * SC],
                            oh_sb[h][:, st * 128 : (st + 1) * 128],
                            wo_sb[:, h, nn * SC : (nn + 1) * SC],
                            start=(h == 0),
                            stop=(h == NQ - 1),
                        )
                ot = btmp.tile([128, S], f32, tag="ost")
                nc.scalar.copy(ot[:], pso[:])
                nc.sync.dma_start(
                    out=out[st * 128 : (st + 1) * 128, :],
                    in_=ot[:],
                )
    return nc


_NC_CACHE = None


def _get_nc():
    global _NC_CACHE
    if _NC_CACHE is None:
        _install_patches()
        _NC_CACHE = _build_nc()
    return _NC_CACHE


# De-interleave permutation: within each head, even dims then odd dims.
_PERM = np.concatenate([np.arange(0, HD, 2), np.arange(1, HD, 2)])

_last_in_maps = None


def kernel(x, Wq, Wk, Wv, Wo, freqs_cos, freqs_sin, start_pos):
    _install_patches()
    import ml_dtypes

    from concourse.bass_utils import run_bass_kernel_spmd

    bf16 = ml_dtypes.bfloat16
    x = np.asarray(x, dtype=np.float32)
    Wq = np.asarray(Wq, dtype=np.float32)
    Wk = np.asarray(Wk, dtype=np.float32)
    Wv = np.asarray(Wv, dtype=np.float32)
    Wo = np.asarray(Wo, dtype=np.float32)
    cos2 = np.ascontiguousarray(np.asarray(freqs_cos, dtype=np.float32).T)
    sin2 = np.ascontiguousarray(np.asarray(freqs_sin, dtype=np.float32).T)

    # Per-head de-interleave of Wq/Wk columns (RoPE pairs -> [re, im] blocks)
    Wq_p = Wq.reshape(HID, N_HEADS, HD)[:, :, _PERM]
    Wk_p = Wk.reshape(HID, N_KV, HD)[:, :, _PERM]

    in_maps = []
    for core in range(8):
        b, t = divmod(core, TP)
        xT_b = np.ascontiguousarray(x[b].T.astype(bf16))
        wq_c = np.ascontiguousarray(
            Wq_p[:, t * NQ : (t + 1) * NQ, :].reshape(HID, NQ * HD).astype(bf16)
        )
        wk_c = np.ascontiguousarray(
            Wk_p[:, t * NKV : (t + 1) * NKV, :].reshape(HID, NKV * HD).astype(bf16)
        )
        wv_c = np.ascontiguousarray(
            Wv.reshape(HID, N_KV, HD)[:, t * NKV : (t + 1) * NKV, :]
            .reshape(HID, NKV * HD)
            .astype(bf16)
        )
        wo_c = np.ascontiguousarray(
            Wo[t * NQ * HD : (t + 1) * NQ * HD, :].astype(bf16)
        )
        in_maps.append(
            {
                "xT": xT_b,
                "wq": wq_c,
                "wk": wk_c,
                "wv": wv_c,
                "wo": wo_c,
                "cos2": cos2,
                "sin2": sin2,
            }
        )

    global _last_in_maps
    _last_in_maps = in_maps
    nc = _get_nc()
    res = run_bass_kernel_spmd(nc, in_maps, list(range(8)))
    outs = [res.results[c]["out"] for c in range(8)]
    full = np.stack(
        [sum(outs[b * TP + t] for t in range(TP)) for b in range(B)]
    ).astype(np.float32)
    return full


# revision 14
# speedup vs baseline: 1.0262x; 1.0086x over previous
"""GQA attention (B=2, S=2048, 16 Q heads / 8 KV heads, head_dim=128, RoPE,
no causal mask) on 8 Trainium2 NeuronCores.

Sharding: DP=2 on batch x TP=4 on heads. Each core computes 4 Q heads /
2 KV heads for one batch element, plus a row-sharded o_proj partial; the
host sums the 4 partials per batch (the "all-reduce").

v2: single fused pass, bf16 operands (f32 PSUM accumulation), everything
SBUF-resident (x loaded once, no Q/K/V DRAM round trips), h-major unit
order so early softmax exps overlap remaining Q projections, paired
[128,1024] exp activations, reciprocal_approx_fast for the softmax divide.
"""
import json
import math
from contextlib import ExitStack

import numpy as np

# ---------------------------------------------------------------------------
# Environment patches (required for the walrus build in this container)
# ---------------------------------------------------------------------------
_PATCHED = False


def _install_patches():
    """1) The walrus here rejects >1 sync wait per instruction; split extra
    waits onto single-wait NoOps inserted before the instruction (engines
    execute their stream in order, so semantics are preserved).
    2) antenv.axon_hooks is missing in this image; shim it so trace=True
    profiling works (used by test harnesses; harmless otherwise)."""
    global _PATCHED
    if _PATCHED:
        return
    _PATCHED = True

    import concourse.bass as bass

    counter = [0]

    def _split_multiwait(bir):
        for func in bir.get("functions", []):
            for block in func.get("blocks", []):
                new_insts = []
                for inst in block.get("instructions", []):
                    si = inst.get("sync_info")
                    waits = (si or {}).get("on_wait") or []
                    if len(waits) > 1:
                        for w in waits[:-1]:
                            counter[0] += 1
                            new_insts.append(
                                {
                                    "debug": inst.get("debug", 0),
                                    "engine": inst.get("engine"),
                                    "ins": [],
                                    "name": f"I-waitsplit-{counter[0]}",
                                    "opcode": "NoOp",
                                    "outs": [],
                                    "sync_info": {"on_wait": [w], "on_update": []},
                                }
                            )
                        si["on_wait"] = [waits[-1]]
                    new_insts.append(inst)
                block["instructions"] = new_insts
        return bir

    orig_to_json_bytes = bass.Bass.to_json_bytes

    def patched_to_json_bytes(self):
        bir = json.loads(orig_to_json_bytes(self))
        return json.dumps(_split_multiwait(bir)).encode()

    bass.Bass.to_json_bytes = patched_to_json_bytes

    # -- NTFF profile hook shim (for trace=True) --
    import sys
    import types

    if "antenv.axon_hooks" not in sys.modules:
        mod = types.ModuleType("antenv.axon_hooks")
        _hook = [None]
        try:
            from trn_agent_boot.trn_boot import _ntff_profile_via_ctypes

            _hook[0] = _ntff_profile_via_ctypes("/opt/axon/libaxon_pjrt.so")
        except Exception:
            pass
        mod.get_axon_ntff_profile_hook = lambda: _hook[0]
        mod.set_axon_ntff_profile_hook = lambda h: _hook.__setitem__(0, h)
        sys.modules["antenv.axon_hooks"] = mod

    # upload_artifacts needs external storage; make it a no-op locally.
    import concourse.bass_utils as bu

    bu.upload_artifacts = lambda tmpdir: str(tmpdir)


# ---------------------------------------------------------------------------
# Problem constants (hardcoded per contest contract)
# ---------------------------------------------------------------------------
B, S, HID = 2, 2048, 2048
N_HEADS, N_KV = 16, 8
HD = 128
TP = 4  # tensor-parallel factor over heads
NQ = N_HEADS // TP  # 4 q heads per core
NKV = N_KV // TP  # 2 kv heads per core
KT = HID // 128  # 16 contraction tiles
ST = S // 128  # 16 sequence tiles of 128
SC = 512  # free-dim chunk
NB = S // SC  # 4 chunks over S
SCALE = 1.0 / math.sqrt(HD)


def _build_nc():
    import concourse.bass as bass
    import concourse.tile as tile
    from concourse import mybir

    f32 = mybir.dt.float32
    bf16 = mybir.dt.bfloat16
    AF = mybir.ActivationFunctionType

    nc = bass.Bass()
    xT = nc.dram_tensor("xT", [HID, S], bf16, kind="ExternalInput")
    wq = nc.dram_tensor("wq", [HID, NQ * HD], bf16, kind="ExternalInput")
    wk = nc.dram_tensor("wk", [HID, NKV * HD], bf16, kind="ExternalInput")
    wv = nc.dram_tensor("wv", [HID, NKV * HD], bf16, kind="ExternalInput")
    wo = nc.dram_tensor("wo", [NQ * HD, HID], bf16, kind="ExternalInput")
    # cos2 rows = [cosT; sinT], sin2 rows = [sinT; cosT]  (stacked host-side)
    cos2 = nc.dram_tensor("cos2", [HD // 2, S], f32, kind="ExternalInput")
    sin2 = nc.dram_tensor("sin2", [HD // 2, S], f32, kind="ExternalInput")
    out = nc.dram_tensor("out", [S, HID], f32, kind="ExternalOutput")

    with tile.TileContext(nc) as tc, ExitStack() as ctx:
        # ---- persistent pools (whole kernel) ----
        const = ctx.enter_context(tc.tile_pool(name="const", bufs=1))
        keep = ctx.enter_context(tc.tile_pool(name="keep", bufs=1))

        ones_f = const.tile([128, 128], f32)
        nc.vector.memset(ones_f[:], 1.0)
        ones_mat = const.tile([128, 128], bf16)
        nc.vector.tensor_copy(ones_mat[:], ones_f[:])
        ones32 = const.tile([128, 32], bf16)
        nc.vector.tensor_copy(ones32[:], ones_f[:, 0:32])

        # persistent SBUF tensors
        kT_sb = [keep.tile([128, S], bf16, tag=f"kT{g}", name=f"kT{g}") for g in range(NKV)]
        qT_sb = [keep.tile([128, S], bf16, tag=f"qT{h}", name=f"qT{h}") for h in range(NQ)]
        v_sb = keep.tile([128, ST, NKV * HD], bf16, tag="v", name="v")
        oh_sb = [keep.tile([128, S], bf16, tag=f"oh{h}", name=f"oh{h}") for h in range(NQ)]

        # ================= Phase A: projections + RoPE =================
        with ExitStack() as actx:
            xpool = actx.enter_context(tc.tile_pool(name="xpool", bufs=1))
            wpool = actx.enter_context(tc.tile_pool(name="wpool", bufs=2))
            wkpool = actx.enter_context(tc.tile_pool(name="wkpool", bufs=1))
            aconst = actx.enter_context(tc.tile_pool(name="aconst", bufs=1))
            rstage = actx.enter_context(tc.tile_pool(name="rstage", bufs=3))
            pspj = actx.enter_context(tc.tile_pool(name="pspj", bufs=4, space="PSUM"))

            # ---- DMA loads (emission order = priority order) ----
            wk_sb = wkpool.tile([128, KT, NKV * HD], bf16, tag="wk")
            nc.gpsimd.dma_start(
                out=wk_sb[:], in_=wk.rearrange("(kt p) d -> p kt d", p=128)
            )
            x_sb = xpool.tile([128, KT, S], bf16, tag="x")
            x_re = xT.rearrange("(kt p) s -> p kt s", p=128)
            for kq in range(4):
                nc.sync.dma_start(
                    out=x_sb[:, 4 * kq : 4 * kq + 4, 0:SC],
                    in_=x_re[:, 4 * kq : 4 * kq + 4, 0:SC],
                )
            wv_sb = aconst.tile([128, KT, NKV * HD], bf16)
            nc.sync.dma_start(
                out=wv_sb[:], in_=wv.rearrange("(kt p) d -> p kt d", p=128)
            )
            nc.sync.dma_start(
                out=x_sb[:, :, SC : 2 * SC], in_=x_re[:, :, SC : 2 * SC]
            )
            cos_sb = aconst.tile([HD // 2, S], f32)
            sin_sb = aconst.tile([HD // 2, S], f32)
            nc.sync.dma_start(out=cos_sb[:], in_=cos2[:, :])
            nc.sync.dma_start(out=sin_sb[:], in_=sin2[:, :])
            for c in range(2, NB):
                nc.sync.dma_start(
                    out=x_sb[:, :, c * SC : (c + 1) * SC],
                    in_=x_re[:, :, c * SC : (c + 1) * SC],
                )
            wq_re = wq.rearrange("(kt p) (h d) -> p kt h d", p=128, d=HD)
            wq_tiles = {}

            def load_wq(h):
                wq_tiles[h] = wpool.tile([128, KT, HD], bf16, tag="wq", name=f"wq{h}")
                nc.sync.dma_start(out=wq_tiles[h][:], in_=wq_re[:, :, h, :])

            load_wq(0)
            load_wq(1)

            # ---- RoPE: ps [128(re/im),512] -> dst[:, c-slice] ----
            # (PSUM inputs may cross partition bases; SBUF+SBUF may not)
            def rope(ps, dst, c0):
                re = ps[0:64, :]
                im = ps[64:128, :]
                cs = cos_sb[:, c0 : c0 + SC]
                sn = sin_sb[:, c0 : c0 + SC]
                t1 = rstage.tile([64, SC], f32, tag="t1")
                t2 = rstage.tile([64, SC], f32, tag="t2")
                t3 = rstage.tile([64, SC], f32, tag="t3")
                t4 = rstage.tile([64, SC], f32, tag="t4")
                nc.vector.tensor_mul(t1[:], re, cs)
                nc.vector.tensor_mul(t2[:], im, sn)
                nc.gpsimd.tensor_sub(dst[0:64, c0 : c0 + SC], t1[:], t2[:])
                nc.vector.tensor_mul(t3[:], re, sn)
                nc.vector.tensor_mul(t4[:], im, cs)
                nc.gpsimd.tensor_add(dst[64:128, c0 : c0 + SC], t3[:], t4[:])

            # ---- K projection + RoPE (all S), then V projection ----
            for c in range(NB):
                c0 = c * SC
                for g in range(NKV):
                    ps = pspj.tile([128, SC], f32, tag="pj")
                    for kt in range(KT):
                        nc.tensor.matmul(
                            ps[:],
                            wk_sb[:, kt, g * HD : (g + 1) * HD],
                            x_sb[:, kt, c0 : c0 + SC],
                            start=(kt == 0),
                            stop=(kt == KT - 1),
                        )
                    rope(ps, kT_sb[g], c0)
                for sti in range(SC // 128):
                    st = c * (SC // 128) + sti
                    ps = pspj.tile([128, SC], f32, tag="pj")
                    for kt in range(KT):
                        nc.tensor.matmul(
                            ps[:, 0 : NKV * HD],
                            x_sb[:, kt, st * 128 : (st + 1) * 128],
                            wv_sb[:, kt, :],
                            start=(kt == 0),
                            stop=(kt == KT - 1),
                        )
                    nc.vector.tensor_copy(v_sb[:, st, :], ps[:, 0 : NKV * HD])

            # ---- Q projection + RoPE, per head ----
            for h in range(NQ):
                if h >= 2:
                    load_wq(h)
                for c in range(NB):
                    c0 = c * SC
                    ps = pspj.tile([128, SC], f32, tag="pj")
                    for kt in range(KT):
                        nc.tensor.matmul(
                            ps[:],
                            wq_tiles[h][:, kt, :],
                            x_sb[:, kt, c0 : c0 + SC],
                            start=(kt == 0),
                            stop=(kt == KT - 1),
                        )
                    rope(ps, qT_sb[h], c0)

        # ================= Phase B: attention units + o_proj =================
        with ExitStack() as bctx:
            bpool = bctx.enter_context(tc.tile_pool(name="bpool", bufs=1))
            btmp = bctx.enter_context(tc.tile_pool(name="btmp", bufs=2))
            ptpool = bctx.enter_context(tc.tile_pool(name="ptpool", bufs=2))
            uctx = ExitStack()
            # PSUM: psmm 2x[128,1024](8KB/p) + pspv 2KB + psden 2KB = 12KB
            psmm = uctx.enter_context(tc.tile_pool(name="psmm", bufs=2, space="PSUM"))
            pspv = uctx.enter_context(tc.tile_pool(name="pspv", bufs=1, space="PSUM"))
            psden = uctx.enter_context(tc.tile_pool(name="psden", bufs=1, space="PSUM"))

            wo_sb = bpool.tile([128, NQ, HID], bf16)
            nc.sync.dma_start(
                out=wo_sb[:], in_=wo.rearrange("(h p) n -> p h n", p=128)
            )

            # ---- attention unit: (h, nb) -> oh_sb[h][:, nb*SC:...] ----
            def unit(h, nb):
                g = h // (NQ // NKV)
                c0 = nb * SC
                pt = ptpool.tile([128, KT, SC], bf16, tag="pt")
                # scores in pairs: two N=512 matmul groups into one
                # [128,1024] psum tile, one exp ACTIVATE per pair
                for p in range(KT // 2):
                    pss = psmm.tile([128, 2 * SC], f32, tag="mm")
                    for j in range(2):
                        kt = 2 * p + j
                        nc.tensor.matmul(
                            pss[:, j * SC : (j + 1) * SC],
                            kT_sb[g][:, kt * 128 : (kt + 1) * 128],
                            qT_sb[h][:, c0 : c0 + SC],
                            start=True,
                            stop=True,
                        )
                    nc.scalar.activation(
                        pt[:, 2 * p : 2 * p + 2, :], pss[:], AF.Exp, scale=SCALE
                    )
                # denominator (broadcast over partitions via ones matmul)
                psdb = psden.tile([128, SC], f32, tag="db")
                for kt in range(KT):
                    nc.tensor.matmul(
                        psdb[:],
                        ones_mat[:],
                        pt[:, kt, :],
                        start=(kt == 0),
                        stop=(kt == KT - 1),
                    )
                recb = btmp.tile([128, SC], f32, tag="recb")
                nc.vector.reciprocal(recb[:], psdb[:])
                # PV
                ppv = pspv.tile([128, SC], f32, tag="pv")
                for kt in range(KT):
                    nc.tensor.matmul(
                        ppv[:],
                        v_sb[:, kt, g * HD : (g + 1) * HD],
                        pt[:, kt, :],
                        start=(kt == 0),
                        stop=(kt == KT - 1),
                    )
                nc.vector.tensor_mul(oh_sb[h][:, c0 : c0 + SC], ppv[:], recb[:])

            for h in range(NQ):
                for nb in range(NB):
                    unit(h, nb)

            uctx.close()
            # ---- o_proj: out[st,:] += sum_h oh[h][:,st].T @ wo[h] ----
            # full-row [128,S] PSUM accumulation: one evict + one 1MB DMA
            # per st-tile, PE-bound instead of eviction-throttled
            psop = bctx.enter_context(tc.tile_pool(name="psop", bufs=2, space="PSUM"))
            for st in range(ST):
                pso = psop.tile([128, S], f32, tag="op")
                for nn in range(NB):
                    for h in range(NQ):
                        nc.tensor.matmul(
                            pso[:, nn * SC : (nn + 1) * SC],
                            oh_sb[h][:, st * 128 : (st + 1) * 128],
                            wo_sb[:, h, nn * SC : (nn + 1) * SC],
                            start=(h == 0),
                            stop=(h == NQ - 1),
                        )
                ot = btmp.tile([128, S], f32, tag="ost")
                nc.scalar.copy(ot[:], pso[:])
                nc.sync.dma_start(
                    out=out[st * 128 : (st + 1) * 128, :],
                    in_=ot[:],
                )
    return nc


_NC_CACHE = None


def _get_nc():
    global _NC_CACHE
    if _NC_CACHE is None:
        _install_patches()
        _NC_CACHE = _build_nc()
    return _NC_CACHE


# De-interleave permutation: within each head, even dims then odd dims.
_PERM = np.concatenate([np.arange(0, HD, 2), np.arange(1, HD, 2)])

_last_in_maps = None


def kernel(x, Wq, Wk, Wv, Wo, freqs_cos, freqs_sin, start_pos):
    _install_patches()
    import ml_dtypes

    from concourse.bass_utils import run_bass_kernel_spmd

    bf16 = ml_dtypes.bfloat16
    x = np.asarray(x, dtype=np.float32)
    Wq = np.asarray(Wq, dtype=np.float32)
    Wk = np.asarray(Wk, dtype=np.float32)
    Wv = np.asarray(Wv, dtype=np.float32)
    Wo = np.asarray(Wo, dtype=np.float32)
    cos2 = np.ascontiguousarray(np.asarray(freqs_cos, dtype=np.float32).T)
    sin2 = np.ascontiguousarray(np.asarray(freqs_sin, dtype=np.float32).T)

    # Per-head de-interleave of Wq/Wk columns (RoPE pairs -> [re, im] blocks)
    Wq_p = Wq.reshape(HID, N_HEADS, HD)[:, :, _PERM]
    Wk_p = Wk.reshape(HID, N_KV, HD)[:, :, _PERM]

    in_maps = []
    for core in range(8):
        b, t = divmod(core, TP)
        xT_b = np.ascontiguousarray(x[b].T.astype(bf16))
        wq_c = np.ascontiguousarray(
            Wq_p[:, t * NQ : (t + 1) * NQ, :].reshape(HID, NQ * HD).astype(bf16)
        )
        wk_c = np.ascontiguousarray(
            Wk_p[:, t * NKV : (t + 1) * NKV, :].reshape(HID, NKV * HD).astype(bf16)
        )
        wv_c = np.ascontiguousarray(
            Wv.reshape(HID, N_KV, HD)[:, t * NKV : (t + 1) * NKV, :]
            .reshape(HID, NKV * HD)
            .astype(bf16)
        )
        wo_c = np.ascontiguousarray(
            Wo[t * NQ * HD : (t + 1) * NQ * HD, :].astype(bf16)
        )
        in_maps.append(
            {
                "xT": xT_b,
                "wq": wq_c,
                "wk": wk_c,
                "wv": wv_c,
                "wo": wo_c,
                "cos2": cos2,
                "sin2": sin2,
            }
        )

    global _last_in_maps
    _last_in_maps = in_maps
    nc = _get_nc()
    res = run_bass_kernel_spmd(nc, in_maps, list(range(8)))
    outs = [res.results[c]["out"] for c in range(8)]
    full = np.stack(
        [sum(outs[b * TP + t] for t in range(TP)) for b in range(B)]
    ).astype(np.float32)
    return full
